# revision 1
# baseline (speedup 1.0000x reference)
"""Trainium2 Bass kernel for nn_MultiHeadAttention (B=2, T=4096, E=512, H=8, dk=dv=64).

Sharding: 8 cores = 2 batches x 4 query-row chunks. Core c handles batch
c//4 and query rows [(c%4)*1024, (c%4+1)*1024). Each core holds the full
K/V of its batch, so it produces complete, final output rows — the host
only slices inputs and concatenates outputs (no cross-core reduction).

Per-core dataflow (all matmuls on PE; default mode keeps fp32 storage and
runs the PE in float32r mode, which needs operands pre-rounded to f32r by
their producing instruction):
  1. PE-transpose Q/K/V input tiles into [E, t] layout (fp32 transpose mode).
  2. Projections: qT[j,t], kT[j,s] (lhsT=W, rhs=X^T), v[s,j] (lhsT=V^T,
     rhs=Wv). Biases folded in as k=1 rank-1 matmuls against a ones row.
  3. Scores computed transposed: S^T[s,t] = kT_h.T @ qT_h (k=dk=64; even/odd
     head pairs sit at partition bases 0/64, enabling PE row-tiling overlap).
  4. exp via ACT (scale=1/8 fused; no max subtraction — scores are N(0,~0.33),
     bounded, so exp is safe in fp32).
  5. PV: out'^T[d,t] accumulated over s-chunks with lhsT = [v_h | ones]
     ([128, 65]) so row 64 accumulates the softmax denominator for free.
  6. Normalize num/den (reciprocal + gpsimd partition-broadcast + DVE mul).
  7. out[t,e] = attnT.T @ Wo + bo, stored in natural layout, DMA'd out.

The softmax s-dimension is processed in NPHASE phases so kT/v fit SBUF;
partial PV accumulators spill to fp32 SBUF between phases.
"""

import os
import sys
from contextlib import ExitStack

for _p in ("/opt/trn_rl_repo",):
    if _p not in sys.path and os.path.isdir(_p):
        sys.path.insert(0, _p)

import numpy as np

import concourse.bass as bass
import concourse.mybir as mybir
import concourse.tile as tile
from concourse import bacc
from concourse.bass_utils import run_bass_kernel_spmd
from concourse.masks import make_identity

F32 = mybir.dt.float32
F32R = mybir.dt.float32r
BF16 = mybir.dt.bfloat16
AF = mybir.ActivationFunctionType

B, T, E, H, D = 2, 4096, 512, 8, 64
TL = T // 4  # query rows per core
S = T  # kv rows per core (full batch)
P = 128
NCORES = 8

CFG = os.environ.get("MHA_CFG", "f32r")  # "f32r" | "bf16" | "f32"


def build_nc(cfg=CFG):
    if cfg == "bf16":
        MDT, NPHASE = BF16, 1
    elif cfg == "f32r":
        MDT, NPHASE = F32R, 2
    elif cfg == "f32":
        MDT, NPHASE = F32, 2
    else:
        raise ValueError(cfg)
    # natural input tiles + PE-transpose dtype. bf16/f32r cast during the
    # SWDGE DMA load; transposes then run at 1.0/1.5 cyc/row instead of 2.0.
    NDT = F32 if cfg == "f32" else MDT
    SH = S // NPHASE  # kv rows per phase
    NSC = SH // P  # s-chunks per phase

    nc = bacc.Bacc(
        "TRN2",
        target_bir_lowering=False,
        debug=False,
        enable_asserts=False,
        num_devices=NCORES,
    )

    # bf16 config: host passes Q/K/V pre-cast to bf16 and we use HWDGE
    # DMA-transpose loads (2-byte dtype only) instead of PE transposes.
    use_dmat = cfg == "bf16"
    IDT = BF16 if use_dmat else F32
    q_d = nc.dram_tensor("q", [TL, E], IDT, kind="ExternalInput").ap()
    k_d = nc.dram_tensor("k", [S, E], IDT, kind="ExternalInput").ap()
    v_d = nc.dram_tensor("v", [S, E], IDT, kind="ExternalInput").ap()
    w_d = {
        n: nc.dram_tensor(n, [E, E], F32, kind="ExternalInput").ap()
        for n in ("wq", "wk", "wv", "wo")
    }
    # bv is folded into bo on the host: attn includes +bv exactly as
    # (sum_s E v)/den + bv, and bv @ Wo is a constant row -> bo' = bv@Wo + bo.
    b_d = {
        n: nc.dram_tensor(n, [1, E], F32, kind="ExternalInput").ap()
        for n in ("bq", "bk", "bo")
    }
    out_d = nc.dram_tensor("out", [TL, E], F32, kind="ExternalOutput").ap()

    def dma_nat(dst, src):
        """DRAM fp32 -> natural-layout tile (NDT)."""
        if NDT == F32:
            nc.sync.dma_start(out=dst, in_=src)
        else:
            nc.gpsimd.dma_start(out=dst, in_=src)

    with tile.TileContext(nc) as tc, ExitStack() as ctx:
        sb = ctx.enter_context(tc.tile_pool(name="sb", bufs=1))
        wpool = ctx.enter_context(tc.tile_pool(name="w", bufs=2))
        if use_dmat:
            xtb = ctx.enter_context(tc.tile_pool(name="xtb", bufs=6))
        else:
            natp = ctx.enter_context(tc.tile_pool(name="nat", bufs=6))
            xtp = ctx.enter_context(tc.tile_pool(name="xt", bufs=8))
        ep = ctx.enter_context(tc.tile_pool(name="ep", bufs=2))
        outp = ctx.enter_context(tc.tile_pool(name="outp", bufs=2))
        rp = ctx.enter_context(tc.tile_pool(name="rp", bufs=2))
        rp2 = ctx.enter_context(tc.tile_pool(name="rp2", bufs=2))
        # PSUM: psA (2 x [128,1024] slots = 4 banks) shared by transposes,
        # projections, scores and the final Wo matmuls; psV (2 x [128,1024]
        # = 4 banks) holds the PV accumulators of one head pair.
        psA = ctx.enter_context(tc.tile_pool(name="psA", bufs=2, space="PSUM"))
        psV = ctx.enter_context(tc.tile_pool(name="psV", bufs=2, space="PSUM"))

        def load_mdt(shape, tag, src_ap, pool):
            """DRAM fp32 -> MDT tile (SWDGE cast-DMA rounds to bf16/f32r)."""
            t = pool.tile(shape, MDT, name=tag, tag=tag)
            if MDT == F32:
                nc.sync.dma_start(out=t[:], in_=src_ap)
            else:
                nc.gpsimd.dma_start(out=t[:], in_=src_ap)
            return t

        if not use_dmat:
            if NDT == F32:
                ident = sb.tile([P, P], F32, name="ident", tag="ident")
                make_identity(nc, ident[:])
            else:
                ident_f = sb.tile([P, P], F32, name="ident_f", tag="ident_f")
                make_identity(nc, ident_f[:])
                ident = sb.tile([P, P], NDT, name="ident", tag="ident")
                nc.vector.tensor_copy(ident[:], ident_f[:])
        ones_f = sb.tile([1, 512], F32, name="ones_f", tag="ones_f")
        nc.gpsimd.memset(ones_f[:], 1.0)
        if MDT == F32:
            ones = ones_f
        else:
            ones = sb.tile([1, 512], MDT, name="ones", tag="ones")
            nc.vector.tensor_copy(ones[:], ones_f[:])
        ones_col_f = sb.tile([P, H, 1], F32, name="ones_col_f", tag="ones_col_f")
        nc.gpsimd.memset(ones_col_f[:], 1.0)
        if MDT == F32:
            ones_col = ones_col_f
        else:
            ones_col = sb.tile([P, H, 1], MDT, name="ones_col", tag="ones_col")
            nc.vector.tensor_copy(ones_col[:], ones_col_f[:])

        bias_t = {n: load_mdt([1, E], n, b_d[n][:], sb) for n in ("bo",)}
        # bq/bk as per-partition columns [128, 4] (jb-indexed) for DVE
        # tensor_scalar bias folding on the projection PSUM->SBUF copy.
        bias_c = {}
        for n in ("bq", "bk"):
            bias_c[n] = sb.tile([P, 4], F32, name=n + "c", tag=n + "c")
            nc.sync.dma_start(
                out=bias_c[n][:], in_=b_d[n].rearrange("o (jb p) -> p (jb o)", p=P)
            )

        qT = sb.tile([P, 4 * TL], MDT, name="qT", tag="qT")  # q^T: jb-block at cols jb*TL
        kT = sb.tile([P, 4 * SH], MDT, name="kT", tag="kT")  # k^T: jb-block at cols jb*SH
        vS = sb.tile([P, NSC * (H * (D + 1))], MDT, name="vS", tag="vS")  # [v_h|1]
        aT = sb.tile([P, 4 * TL], MDT, name="aT", tag="aT")  # attn-out^T (normalized)
        accn = sb.tile([P, 4 * TL], F32, name="accn", tag="accn")  # PV numerator accum
        # den accum: pair (h, tc2) lives at partition 32*(h%4), col block
        # (h//4)*1024 + tc2*512 (DVE writes need 32-aligned start partitions).
        # Pre-fill with 1.0 so the full-tile reciprocal stays finite.
        accd = sb.tile([P, 2048], F32, name="accd", tag="accd")
        nc.gpsimd.memset(accd[:], 1.0)

        def den_slice(t, h, tc2):
            p0 = 32 * (h % 4)
            c0 = (h // 4) * 1024 + tc2 * 512
            return t[p0 : p0 + 1, c0 : c0 + 512]

        def load_w(name):
            return load_mdt(
                [P, 4, 512], "w", w_d[name].rearrange("(eb p) j -> p eb j", p=P), wpool
            )

        def transpose_group(x_dram, row0):
            """Load 4 natural tiles [128, 512] starting at x_dram[row0] and
            return 4 transposed tiles: res[eb] = X[row0:row0+512, eb*128:+128].T
            """
            nats = []
            for i in range(4):
                nt = natp.tile([P, E], NDT, name="nat", tag="nat")
                dma_nat(nt[:], x_dram[row0 + i * P : row0 + (i + 1) * P, :])
                nats.append(nt)
            res = []
            for eb in range(4):
                pt = psA.tile([P, 512], NDT, name="psA", tag="psA")
                for i in range(4):
                    nc.tensor.transpose(
                        pt[:, i * P : (i + 1) * P],
                        nats[i][:, eb * P : (eb + 1) * P],
                        ident[:],
                    )
                xt = xtp.tile([P, 512], MDT, name="xt", tag="xt")
                nc.vector.tensor_copy(xt[:], pt[:])
                res.append(xt)
            return res

        def phase_tiles(x_dram, row0, nrows):
            """Return fn(g) -> 4 e-block views [128, 512] of X^T covering
            rows [row0 + g*512, row0 + (g+1)*512)."""
            if use_dmat:
                big = []
                for eb in range(4):
                    xt = xtb.tile([P, nrows], MDT, name="xtb", tag="xtb")
                    nc.sync.dma_start(
                        out=xt[:],
                        in_=x_dram[row0 : row0 + nrows, eb * P : (eb + 1) * P],
                        transpose=True,
                    )
                    big.append(xt)
                return lambda g: [
                    big[eb][:, g * 512 : (g + 1) * 512] for eb in range(4)
                ]
            return lambda g: [t[:] for t in transpose_group(x_dram, row0 + g * 512)]

        # ---- Q: transpose + projection -> qT ----
        wq_t = load_w("wq")
        get_q = phase_tiles(q_d, 0, TL)
        for g in range(TL // 512):
            qts = get_q(g)
            for jb in range(4):
                pt = psA.tile([P, 512], F32, name="psA", tag="psA")
                for eb in range(4):
                    nc.tensor.matmul(
                        pt[:],
                        wq_t[:, eb, jb * P : (jb + 1) * P],
                        qts[eb],
                        start=(eb == 0),
                        stop=(eb == 3),
                    )
                nc.vector.tensor_scalar_add(
                    qT[:, jb * TL + g * 512 : jb * TL + (g + 1) * 512],
                    pt[:],
                    bias_c["bq"][:, jb : jb + 1],
                )

        for ph in range(NPHASE):
            s0 = ph * SH
            # ---- K: transpose + projection -> kT (phase-local) ----
            wk_t = load_w("wk")
            get_k = phase_tiles(k_d, s0, SH)
            for g in range(SH // 512):
                kts = get_k(g)
                for jb in range(4):
                    pt = psA.tile([P, 512], F32, name="psA", tag="psA")
                    for eb in range(4):
                        nc.tensor.matmul(
                            pt[:],
                            wk_t[:, eb, jb * P : (jb + 1) * P],
                            kts[eb],
                            start=(eb == 0),
                            stop=(eb == 3),
                        )
                    nc.vector.tensor_scalar_add(
                        kT[:, jb * SH + g * 512 : jb * SH + (g + 1) * 512],
                        pt[:],
                        bias_c["bk"][:, jb : jb + 1],
                    )
            # ---- V: transpose + projection -> vS (phase-local) ----
            wv_t = load_w("wv")
            get_v = phase_tiles(v_d, s0, SH)
            for g in range(SH // 512):
                vts = get_v(g)
                for i in range(4):
                    sc = g * 4 + i
                    pt = psA.tile([P, 512], F32, name="psA", tag="psA")
                    for eb in range(4):
                        nc.tensor.matmul(
                            pt[:],
                            vts[eb][:, i * P : (i + 1) * P],
                            wv_t[:, eb, :],
                            start=(eb == 0),
                            stop=(eb == 3),
                        )
                    dst = vS[:, sc * 520 : (sc + 1) * 520].rearrange(
                        "p (h x) -> p h x", x=D + 1
                    )
                    nc.vector.tensor_copy(
                        dst[:, :, 0:D], pt[:].rearrange("p (h d) -> p h d", d=D)
                    )
                    if ph == 0:
                        nc.vector.tensor_copy(dst[:, :, D : D + 1], ones_col[:])

            # ---- attention over this phase's s-range ----
            for hp in range(4):
                heads = (2 * hp, 2 * hp + 1)
                pv = {h: psV.tile([P, TL], F32, name="pv", tag="pv") for h in heads}
                for sc in range(NSC):
                    ets = {}
                    for h in heads:
                        r0 = (h % 2) * 64
                        st = psA.tile([P, TL], F32, name="psA", tag="psA")
                        for tc2 in range(2):
                            nc.tensor.matmul(
                                st[:, tc2 * 512 : (tc2 + 1) * 512],
                                kT[
                                    r0 : r0 + 64,
                                    (h // 2) * SH
                                    + sc * P : (h // 2) * SH
                                    + (sc + 1) * P,
                                ],
                                qT[
                                    r0 : r0 + 64,
                                    (h // 2) * TL
                                    + tc2 * 512 : (h // 2) * TL
                                    + (tc2 + 1) * 512,
                                ],
                                start=True,
                                stop=True,
                            )
                        et = ep.tile([P, TL], MDT, name="E", tag="E")
                        nc.scalar.activation(et[:], st[:], AF.Exp, scale=0.125)
                        ets[h] = et
                    for h in heads:
                        for tc2 in range(2):
                            nc.tensor.matmul(
                                pv[h][0 : D + 1, tc2 * 512 : (tc2 + 1) * 512],
                                vS[
                                    :,
                                    sc * 520 + h * (D + 1) : sc * 520
                                    + (h + 1) * (D + 1),
                                ],
                                ets[h][:, tc2 * 512 : (tc2 + 1) * 512],
                                start=(sc == 0),
                                stop=(sc == NSC - 1),
                            )
                # flush PV accumulators to SBUF
                for h in heads:
                    r0 = (h % 2) * 64
                    for tc2 in range(2):
                        nsl = accn[
                            r0 : r0 + 64,
                            (h // 2) * TL + tc2 * 512 : (h // 2) * TL + (tc2 + 1) * 512,
                        ]
                        dsl = den_slice(accd, h, tc2)
                        pn = pv[h][0:D, tc2 * 512 : (tc2 + 1) * 512]
                        pd = pv[h][D : D + 1, tc2 * 512 : (tc2 + 1) * 512]
                        if ph == 0:
                            nc.vector.tensor_copy(nsl, pn)
                            nc.vector.tensor_copy(dsl, pd)
                        else:
                            nc.vector.tensor_add(nsl, nsl, pn)
                            nc.vector.tensor_add(dsl, dsl, pd)

                # ---- per-pair normalization (final phase): aT = accn/accd.
                # Overlaps with the next pair's attention; den reciprocal is
                # broadcast across partitions via a k=1 PE matmul into PSUM.
                if ph == NPHASE - 1:
                    pr0 = 64 * (hp % 2)
                    pc0 = (hp // 2) * 1024
                    rdp = rp2.tile([64, 1024], F32, name="rdp", tag="rdp")
                    nc.vector.reciprocal(
                        rdp[:], accd[pr0 : pr0 + 64, pc0 : pc0 + 1024]
                    )
                    for h in heads:
                        rr = 32 * (h % 2)
                        r0h = (h % 2) * 64
                        for tc2 in range(2):
                            rst = rp.tile([1, 512], MDT, name="rst", tag="rst")
                            nc.vector.tensor_copy(
                                rst[:], rdp[rr : rr + 1, tc2 * 512 : (tc2 + 1) * 512]
                            )
                            repp = psA.tile([P, 512], F32, name="psA", tag="psA")
                            nc.tensor.matmul(
                                repp[0:64, :],
                                ones[0:1, 0:64],
                                rst[:],
                                start=True,
                                stop=True,
                            )
                            sl = slice(
                                (h // 2) * TL + tc2 * 512,
                                (h // 2) * TL + (tc2 + 1) * 512,
                            )
                            nc.vector.tensor_mul(
                                aT[r0h : r0h + 64, sl],
                                accn[r0h : r0h + 64, sl],
                                repp[0:64, :],
                            )

        # ---- output projection: out = attn^T.T @ Wo + bo ----
        wo_t = load_w("wo")
        for mt in range(TL // P):
            pt = psA.tile([P, 512], F32, name="psA", tag="psA")
            for jb in range(4):
                nc.tensor.matmul(
                    pt[:],
                    aT[:, jb * TL + mt * P : jb * TL + (mt + 1) * P],
                    wo_t[:, jb, :],
                    start=(jb == 0),
                    stop=False,
                )
            nc.tensor.matmul(
                pt[:],
                ones[0:1, 0:P],
                bias_t["bo"][:],
                start=False,
                stop=True,
            )
            ot = outp.tile([P, 512], F32, name="out", tag="out")
            nc.vector.tensor_copy(ot[:], pt[:])
            nc.sync.dma_start(out_d[mt * P : (mt + 1) * P, :], ot[:])

    nc.compile()
    return nc


_NC_CACHE = {}


def get_nc(cfg=CFG):
    if cfg not in _NC_CACHE:
        _NC_CACHE[cfg] = build_nc(cfg)
    return _NC_CACHE[cfg]


def make_in_maps(Q, K, V, Wq, bq, Wk, bk, Wv, bv, Wo, bo, cfg=CFG):
    f = lambda x: np.ascontiguousarray(np.asarray(x, dtype=np.float32))
    Q, K, V = f(Q), f(K), f(V)
    if cfg == "bf16":
        import ml_dtypes

        g = lambda x: np.ascontiguousarray(x.astype(ml_dtypes.bfloat16))
        Q, K, V = g(Q), g(K), g(V)
    # bv folds exactly into bo: attn_out = (sum_s E v)/den + bv, and the
    # constant bv row maps through Wo -> bo' = bv @ Wo + bo.
    bo2 = f(bv).reshape(1, E) @ f(Wo) + f(bo).reshape(1, E)
    shared = {
        "wq": f(Wq),
        "wk": f(Wk),
        "wv": f(Wv),
        "wo": f(Wo),
        "bq": f(bq).reshape(1, E),
        "bk": f(bk).reshape(1, E),
        "bo": f(bo2),
    }
    in_maps = []
    for c in range(NCORES):
        b, tq = divmod(c, 4)
        in_maps.append(
            {
                "q": np.ascontiguousarray(Q[b, tq * TL : (tq + 1) * TL, :]),
                "k": K[b],
                "v": V[b],
                **shared,
            }
        )
    return in_maps


def assemble(results):
    out = np.empty((B, T, E), np.float32)
    for c in range(NCORES):
        b, tq = divmod(c, 4)
        out[b, tq * TL : (tq + 1) * TL, :] = results[c]["out"]
    return out


def kernel(Q, K, V, Wq, bq, Wk, bk, Wv, bv, Wo, bo):
    nc = get_nc()
    in_maps = make_in_maps(Q, K, V, Wq, bq, Wk, bk, Wv, bv, Wo, bo)
    res = run_bass_kernel_spmd(nc, in_maps, list(range(NCORES)))
    return assemble(res.results)

